# revision 3
# baseline (speedup 1.0000x reference)
"""Masked dot-product attention (B=64, L=1024, D=64, fp32) on 8 NeuronCores.

Strategy (data-parallel over batch, per the sharding hint):
  - Batches are sorted by valid_len (descending) and dealt round-robin to the
    8 cores so every core gets one batch from each of 8 "rank groups"; the
    per-slot key-block loop count is baked at build time as the max over that
    slot's rank group.  Key blocks that are entirely masked are never computed.
  - Scores are computed transposed, S^T[k, q] = K @ Q^T, via
    matmul(lhsT=K^T_slice, rhs=Q^T) so that the softmax axis (k) lands on the
    partition dim.  Q and K are passed pre-transposed [D, L] per batch (host
    layout choice at shard time; there is no 4-byte DMA transpose on TRN2).
  - The sequence mask is fused into the exp: ScalarE computes
    P^T = exp(S^T/8 + bias_k) with a per-partition bias column that is 0 for
    valid keys and -1e6 for masked keys (exp underflows to exactly 0).
  - AV uses V augmented with a ones column: O'^T = [V | 1]^T @ P^T, so row 64
    of the accumulator is the softmax denominator for free.
  - Normalization: PE broadcasts l across partitions with a ones-column
    matmul, VectorE does reciprocal + multiply.
All matmuls run in float32r (~1.2e-4 relative error, full PE rate).
"""

import math
from contextlib import ExitStack

import numpy as np

import concourse.bass as bass
import concourse.bacc as bacc
import concourse.mybir as mybir
import concourse.tile as tile
from concourse.bass_utils import run_bass_kernel_spmd

F32 = mybir.dt.float32
F32R = mybir.dt.float32r
EXP = mybir.ActivationFunctionType.Exp

B, L, D = 64, 1024, 64
N_CORES = 8
SLOTS = B // N_CORES  # batches per core
KB = 128              # key-block size (partition dim of S^T)
N_KB = L // KB        # max key blocks
QH = 512              # q chunk per matmul (fp32 moving-operand max)
NEG = -1000000.0


def build_kernel(counts):
    """counts[s] = number of 128-wide key blocks to process for slot s."""
    nc = bacc.Bacc()

    qt_d = nc.dram_tensor("qt", [SLOTS, D, L], F32R, kind="ExternalInput")
    kt_d = nc.dram_tensor("kt", [SLOTS, D, L], F32R, kind="ExternalInput")
    v_d = nc.dram_tensor("v", [SLOTS, L, D + 1], F32R, kind="ExternalInput")
    ones_d = nc.dram_tensor("ones", [1, D], F32R, kind="ExternalInput")
    bias_d = nc.dram_tensor("bias", [KB, SLOTS * N_KB], F32, kind="ExternalInput")
    out_d = nc.dram_tensor("out", [SLOTS, D, L], F32, kind="ExternalOutput")

    with tile.TileContext(nc) as tc, ExitStack() as ctx:
        const_pool = ctx.enter_context(tc.tile_pool(name="const", bufs=1))
        qk_pool = ctx.enter_context(tc.tile_pool(name="qk", bufs=2))
        v_pool = ctx.enter_context(tc.tile_pool(name="v", bufs=2))
        p_pool = ctx.enter_context(tc.tile_pool(name="p", bufs=3))
        ep_pool = ctx.enter_context(tc.tile_pool(name="ep", bufs=2))
        out_pool = ctx.enter_context(tc.tile_pool(name="out", bufs=2))
        psum_s = ctx.enter_context(
            tc.tile_pool(name="psum_s", bufs=2, space="PSUM")
        )
        psum_o = ctx.enter_context(
            tc.tile_pool(name="psum_o", bufs=2, space="PSUM")
        )

        bias_t = const_pool.tile([KB, SLOTS * N_KB], F32)
        nc.sync.dma_start(bias_t[:], bias_d[:])
        ones_t = const_pool.tile([1, D], F32R)
        nc.sync.dma_start(ones_t[:], ones_d[:])

        for pair in range(SLOTS // 2):
            n_max = counts[2 * pair]
            # Two batches packed on the partition dim: even batch in
            # partitions 0-63, odd batch in 64-127.
            qt_t = qk_pool.tile([2 * D, L], F32R, tag="qt")
            nc.sync.dma_start(
                qt_t[:],
                qt_d[2 * pair : 2 * pair + 2].rearrange("b d l -> (b d) l"),
            )
            kt_t = qk_pool.tile([2 * D, L], F32R, tag="kt")
            nc.sync.dma_start(
                kt_t[:, : n_max * KB],
                kt_d[2 * pair : 2 * pair + 2].rearrange("b d l -> (b d) l")[
                    :, : n_max * KB
                ],
            )

            for half in range(2):
                s = 2 * pair + half
                n_kb = counts[s]
                rows = slice(D * half, D * half + D)

                v_t = v_pool.tile([KB, N_KB, D + 1], F32R)
                nc.sync.dma_start(
                    v_t[:, :n_kb, :],
                    v_d[s].rearrange("(n p) d -> p n d", p=KB)[:, :n_kb, :],
                )

                o_ps = psum_o.tile([D + 1, L], F32, tag="o")
                for kb in range(n_kb):
                    s_ps = psum_s.tile([KB, L], F32, tag="s")
                    for qh in range(L // QH):
                        nc.tensor.matmul(
                            s_ps[:, qh * QH : (qh + 1) * QH],
                            kt_t[rows, kb * KB : (kb + 1) * KB],
                            qt_t[rows, qh * QH : (qh + 1) * QH],
                            start=True,
                            stop=True,
                        )
                    p_t = p_pool.tile([KB, L], F32R)
                    nc.scalar.activation(
                        p_t[:],
                        s_ps[:],
                        EXP,
                        bias=bias_t[:, s * N_KB + kb : s * N_KB + kb + 1],
                        scale=1.0 / math.sqrt(D),
                    )
                    for qh in range(L // QH):
                        nc.tensor.matmul(
                            o_ps[:, qh * QH : (qh + 1) * QH],
                            v_t[:, kb, :],
                            p_t[:, qh * QH : (qh + 1) * QH],
                            start=(kb == 0),
                            stop=(kb == n_kb - 1),
                        )

                # Epilogue: divide O' rows 0..63 by the denominator row 64.
                l_sb = ep_pool.tile([1, L], F32R, tag="l")
                nc.vector.tensor_copy(l_sb[:], o_ps[D : D + 1, :])
                bc_ps = psum_s.tile([D, L], F32, tag="s")
                for qh in range(L // QH):
                    nc.tensor.matmul(
                        bc_ps[:, qh * QH : (qh + 1) * QH],
                        ones_t[:],
                        l_sb[:, qh * QH : (qh + 1) * QH],
                        start=True,
                        stop=True,
                    )
                rec_sb = ep_pool.tile([D, L], F32, tag="rec")
                nc.vector.reciprocal(rec_sb[:], bc_ps[:])
                out_sb = out_pool.tile([D, L], F32)
                nc.vector.tensor_tensor(
                    out_sb[:], o_ps[:D, :], rec_sb[:], op=mybir.AluOpType.mult
                )
                nc.sync.dma_start(out_d[s], out_sb[:])

    nc.finalize()
    return nc


_NC_CACHE: dict[tuple, object] = {}


def _prepare(queries, keys, values, valid_lens):
    queries = np.ascontiguousarray(queries, dtype=np.float32)
    keys = np.ascontiguousarray(keys, dtype=np.float32)
    values = np.ascontiguousarray(values, dtype=np.float32)
    valid_lens = np.asarray(valid_lens)
    assert queries.shape == (B, L, D), queries.shape
    vl = valid_lens.astype(np.int64)

    # Sort batches by valid_len descending; slot s on core c gets the batch
    # of rank s*8 + c.  Each slot's loop count covers the max valid_len in
    # its rank group, so one instruction stream fits all cores.
    order = np.argsort(-vl, kind="stable")
    counts = tuple(
        max(1, math.ceil(int(vl[order[s * N_CORES]]) / KB)) for s in range(SLOTS)
    )
    # Pairs share a K^T tile sized by the even slot; counts are descending.
    nc = _NC_CACHE.get(counts)
    if nc is None:
        nc = build_kernel(counts)
        _NC_CACHE[counts] = nc

    col = np.arange(L)
    in_maps = []
    for c in range(N_CORES):
        batch_idx = [int(order[s * N_CORES + c]) for s in range(SLOTS)]
        qt = np.ascontiguousarray(
            queries[batch_idx].transpose(0, 2, 1)
        )  # [SLOTS, D, L]
        kt = np.ascontiguousarray(keys[batch_idx].transpose(0, 2, 1))
        v = np.concatenate(
            [values[batch_idx], np.ones((SLOTS, L, 1), np.float32)], axis=2
        )
        bias = np.zeros((KB, SLOTS * N_KB), dtype=np.float32)
        for s in range(SLOTS):
            mask = (col >= vl[batch_idx[s]]).astype(np.float32) * NEG  # [L]
            bias[:, s * N_KB : (s + 1) * N_KB] = mask.reshape(N_KB, KB).T
        in_maps.append(
            {
                "qt": qt,
                "kt": kt,
                "v": v,
                "bias": bias,
                "ones": np.ones((1, D), np.float32),
            }
        )
    return nc, in_maps, order


def _unshard(res, order):
    out = np.empty((B, L, D), dtype=np.float32)
    for c in range(N_CORES):
        o = res.results[c]["out"]  # [SLOTS, D, L]
        for s in range(SLOTS):
            out[int(order[s * N_CORES + c])] = o[s].T
    return out


def kernel(queries, keys, values, valid_lens):
    nc, in_maps, order = _prepare(queries, keys, values, valid_lens)
    res = run_bass_kernel_spmd(nc, in_maps, core_ids=list(range(N_CORES)))
    return _unshard(res, order)


def trace_run(queries, keys, values, valid_lens):
    """Like kernel() but traced; returns BassKernelResults (for test.py)."""
    nc, in_maps, order = _prepare(queries, keys, values, valid_lens)
    res = run_bass_kernel_spmd(
        nc, in_maps, core_ids=list(range(N_CORES)), trace=True
    )
    res.full_output = _unshard(res, order)
    return res


# revision 26
# speedup vs baseline: 17.0785x; 17.0785x over previous
"""Masked dot-product attention (B=64, L=1024, D=64, fp32) on 8 NeuronCores.

Strategy (data-parallel over batch, per the sharding hint):
  - Batches are sorted by valid_len (descending) and dealt round-robin to the
    8 cores so every core gets one batch from each of 8 "rank groups"; the
    per-slot key-block loop count is baked at build time as the max over that
    slot's rank group.  Key blocks that are entirely masked are never computed.
  - Scores are computed transposed, S^T[k, q] = K @ Q^T, via
    matmul(lhsT=K^T_slice, rhs=Q^T) so that the softmax axis (k) lands on the
    partition dim.  Q and K are passed pre-transposed [D, L] per batch (host
    layout choice at shard time; there is no 4-byte DMA transpose on TRN2).
  - The sequence mask is fused into the exp: ScalarE computes
    P^T = exp(S^T/8 + bias_k) with a per-partition bias column that is 0 for
    valid keys and -1e6 for masked keys (exp underflows to exactly 0).
  - AV uses V augmented with a ones column: O'^T = [V | 1]^T @ P^T, so row 64
    of the accumulator is the softmax denominator for free.
  - Normalization: PE broadcasts l across partitions with a ones-column
    matmul, VectorE does reciprocal + multiply.
All matmuls run in float32r (~1.2e-4 relative error, full PE rate).

Scheduling notes (the in-order engine streams make emission order matter):
  - kb loop is software-pipelined: QK(kb+1) is emitted before AV(kb) so PE
    never parks behind an AV that waits on ScalarE's exp.
  - Pair/slot input DMAs are prefetched one slot ahead; the first pair's
    loads are split so the first QK only waits on ~300KB.
  - The divide epilogue is deferred into the next slot's loop and split into
    independent q-halves to shorten the end-of-kernel serial chain.
"""

import math
from contextlib import ExitStack

import numpy as np

import concourse.bass as bass
import concourse.bacc as bacc
import concourse.mybir as mybir
import concourse.tile as tile
from concourse.bass_utils import run_bass_kernel_spmd

F32 = mybir.dt.float32
F32R = mybir.dt.float32r
EXP = mybir.ActivationFunctionType.Exp

B, L, D = 64, 1024, 64
N_CORES = 8
SLOTS = B // N_CORES  # batches per core
KB = 128              # key-block size (partition dim of S^T)
N_KB = L // KB        # max key blocks
QH = 512              # q chunk per matmul (fp32 moving-operand max)
NQH = L // QH
NEG = -1000000.0


def build_kernel(counts):
    """counts[s] = number of 128-wide key blocks to process for slot s."""
    nc = bacc.Bacc()

    qt_d = nc.dram_tensor("qt", [SLOTS, D, L], F32R, kind="ExternalInput")
    kt_d = nc.dram_tensor("kt", [SLOTS, D, L], F32R, kind="ExternalInput")
    v_d = nc.dram_tensor("v", [SLOTS, L, D + 1], F32R, kind="ExternalInput")
    bias_d = nc.dram_tensor("bias", [KB, SLOTS * N_KB], F32, kind="ExternalInput")
    out_d = nc.dram_tensor("out", [SLOTS, D, L], F32, kind="ExternalOutput")

    with tile.TileContext(nc) as tc, ExitStack() as ctx:
        const_pool = ctx.enter_context(tc.tile_pool(name="const", bufs=1))
        qk_pool = ctx.enter_context(tc.tile_pool(name="qk", bufs=3))
        v_pool = ctx.enter_context(tc.tile_pool(name="v", bufs=4))
        p_pool = ctx.enter_context(tc.tile_pool(name="p", bufs=4))
        ep_pool = ctx.enter_context(tc.tile_pool(name="ep", bufs=3))
        out_pool = ctx.enter_context(tc.tile_pool(name="out", bufs=2))
        psum_s = ctx.enter_context(tc.tile_pool(name="psum_s", bufs=2, space="PSUM"))
        psum_o = ctx.enter_context(tc.tile_pool(name="psum_o", bufs=2, space="PSUM"))

        bias_t = const_pool.tile([KB, SLOTS * N_KB], F32)
        warm_t = const_pool.tile([1, 1], F32)
        ones_t = const_pool.tile([1, D], F32R)

        pair_tiles: dict[int, tuple] = {}
        v_tiles: dict[int, object] = {}
        pair_order = [1, 2, 3, 0]  # big pair last: tail epilogues hide in its long loops
        slot_order = [2 * p + h for p in pair_order for h in range(2)]
        next_pair = {pair_order[i]: pair_order[i + 1] for i in range(len(pair_order) - 1)}
        next_slot = {slot_order[i]: slot_order[i + 1] for i in range(len(slot_order) - 1)}

        def load_pair(p):
            if p in pair_tiles:
                return
            n_max = counts[2 * p]
            # Two batches packed on the partition dim: even batch in
            # partitions 0-63, odd batch in 64-127.
            qt_t = qk_pool.tile([2 * D, L], F32R, tag="qt", name="qt_t")
            kt_t = qk_pool.tile([2 * D, L], F32R, tag="kt", name="kt_t")
            src_q = qt_d[2 * p : 2 * p + 2].rearrange("b d l -> (b d) l")
            src_k = kt_d[2 * p : 2 * p + 2].rearrange("b d l -> (b d) l")
            if not pair_tiles:
                # Piecewise: the first slot's kb-0 QKs only wait on the kt
                # head block + their own 64 qt rows (~320KB, 2 gens).
                nc.sync.dma_start(kt_t[:, :KB], src_k[:, :KB])
                nc.sync.dma_start(qt_t[:D, :], src_q[:D, :])
                nc.sync.dma_start(qt_t[D:, :], src_q[D:, :])
                if n_max > 1:
                    nc.sync.dma_start(
                        kt_t[:, KB : n_max * KB], src_k[:, KB : n_max * KB]
                    )
            else:
                nc.sync.dma_start(qt_t[:], src_q)
                nc.sync.dma_start(kt_t[:, : n_max * KB], src_k[:, : n_max * KB])
            pair_tiles[p] = (qt_t, kt_t)

        def load_v(s):
            if s in v_tiles:
                return
            n_kb = counts[s]
            v_t = v_pool.tile([KB, N_KB, D + 1], F32R, name="v_t")
            nc.gpsimd.dma_start(
                v_t[:, :n_kb, :],
                v_d[s].rearrange("(n p) d -> p n d", p=KB)[:, :n_kb, :],
            )
            v_tiles[s] = v_t

        def qk(s_ps, rows, kt_t, qt_t, kb):
            for qh in range(NQH):
                nc.tensor.matmul(
                    s_ps[:, qh * QH : (qh + 1) * QH],
                    kt_t[rows, kb * KB : (kb + 1) * KB],
                    qt_t[rows, qh * QH : (qh + 1) * QH],
                    start=True,
                    stop=True,
                )

        def make_tail(s, o_ps, rec_b, qh):
            # Deferred epilogue part B for one q-half: divide and store.
            c0, c1 = qh * QH, (qh + 1) * QH

            last = s == slot_order[-1]

            def tail():
                out_sb = out_pool.tile([D, QH], F32, name="out_sb")
                nch = 1
                cw = QH // nch
                for ch in range(nch):
                    nc.vector.tensor_tensor(
                        out_sb[:, ch * cw : (ch + 1) * cw],
                        o_ps[:D, c0 + ch * cw : c0 + (ch + 1) * cw],
                        rec_b[:, c0 + ch * cw : c0 + (ch + 1) * cw],
                        op=mybir.AluOpType.mult,
                    )
                    nc.sync.dma_start(
                        out_d[s][:, c0 + ch * cw : c0 + (ch + 1) * cw],
                        out_sb[:, ch * cw : (ch + 1) * cw],
                    )

            return tail

        load_pair(pair_order[0])
        # bias rides the SWDGE path so the first exp isn't queued behind
        # the HWDGE input loads.
        nc.gpsimd.dma_start(bias_t[:], bias_d[:])
        # Warm the exp table set while the first DMAs run; also build a
        # ones row (exp of 0 * bias) for the tail's PE broadcast.
        nc.scalar.activation(warm_t[:], bias_t[0:1, 0:1], EXP)
        nc.scalar.activation(ones_t[:], bias_t[0:1, :D], EXP, scale=0.0)
        load_v(slot_order[0])
        pending_tails: list = []
        for s in slot_order:
            pair, half = divmod(s, 2)
            n_kb = counts[s]
            rows = slice(D * half, D * half + D)
            qt_t, kt_t = pair_tiles[pair]
            v_t = v_tiles[s]


            # Prefetch upcoming inputs (pairs up to two ahead).
            if s in next_slot:
                load_v(next_slot[s])
                if next_slot[s] in next_slot:
                    load_v(next_slot[next_slot[s]])
            if half == 0 and pair in next_pair:
                load_pair(next_pair[pair])
            if half == 1 and pair in next_pair and next_pair[pair] in next_pair:
                load_pair(next_pair[next_pair[pair]])

            o_ps = psum_o.tile([D + 1, L], F32, tag="o", name="o_ps")

            def av(kb, p_t):
                for qh in range(NQH):
                    nc.tensor.matmul(
                        o_ps[:, qh * QH : (qh + 1) * QH],
                        v_t[:, kb, :],
                        p_t[:, qh * QH : (qh + 1) * QH],
                        start=(kb == 0),
                        stop=(kb == n_kb - 1),
                    )

            # Software-pipelined kb loop, depth 2: the PE stream is
            # QK(kb+1), AV(kb-1) — AV only consumes an exp finished a full
            # iteration ago, so PE never parks on ScalarE.
            s_tiles = {0: psum_s.tile([KB, L], F32, tag="s", name="s_ps")}
            qk(s_tiles[0], rows, kt_t, qt_t, 0)
            p_tiles = {}
            for kb in range(n_kb):
                if kb + 1 < n_kb:
                    s_tiles[kb + 1] = psum_s.tile([KB, L], F32, tag="s", name="s_ps")
                    qk(s_tiles[kb + 1], rows, kt_t, qt_t, kb + 1)
                # Emit the previous slot's deferred epilogue pieces early in
                # this slot's loop (rec_b is ready by then).
                if pending_tails and kb == min(2, n_kb - 1):
                    for t in pending_tails:
                        t()
                    pending_tails = []
                s_ps = s_tiles.pop(kb)
                p_tiles[kb] = p_pool.tile([KB, L], F32R, name="p_t")
                nc.scalar.activation(
                    p_tiles[kb][:],
                    s_ps[:],
                    EXP,
                    bias=bias_t[:, s * N_KB + kb : s * N_KB + kb + 1],
                    scale=1.0 / math.sqrt(D),
                )
                if kb >= 1:
                    av(kb - 1, p_tiles.pop(kb - 1))
            av(n_kb - 1, p_tiles.pop(n_kb - 1))

            # Epilogue part A, chunked per q-half so the tail pipeline
            # overlaps: reciprocal of the denominator row (VectorE, PSUM ->
            # SBUF), then replicate across 64 partitions with a step-0
            # free-dim SBUF->SBUF DMA on the (idle) SWDGE path.
            rdt = F32R if s == slot_order[-1] else F32
            rec_row = ep_pool.tile([1, L], rdt, tag="l", name="rec_row")
            rec_b = ep_pool.tile([D, L], rdt, tag="rec", name="rec_b")
            for qh in range(NQH):
                c0, c1 = qh * QH, (qh + 1) * QH
                with nc.allow_low_precision("f32r label for PE-broadcast tail"):
                    nc.vector.reciprocal(rec_row[:, c0:c1], o_ps[D : D + 1, c0:c1])
                if s == slot_order[-1]:
                    # Tail: PE broadcast + VectorE copy (low latency; the
                    # score PSUM banks are free by now).
                    bc_ps = psum_s.tile([D, QH], F32, tag="s", name="bc_ps")
                    nc.tensor.matmul(
                        bc_ps[:],
                        ones_t[:],
                        rec_row[:, c0:c1],
                        start=True,
                        stop=True,
                    )
                    nc.vector.tensor_copy(rec_b[:, c0:c1], bc_ps[:])
                else:
                    row_ap = rec_row[:, c0:c1]
                    bcast_src = bass.AP(
                        row_ap.tensor, row_ap.offset,
                        [list(row_ap.ap)[0], [0, D]] + list(row_ap.ap)[1:],
                    )
                    nc.gpsimd.dma_start(rec_b[:, c0:c1], bcast_src)
            pending_tails = [make_tail(s, o_ps, rec_b, qh) for qh in range(NQH)]

        for t in pending_tails:
            t()

    nc.finalize()
    return nc


_NC_CACHE: dict[tuple, object] = {}


def _prepare(queries, keys, values, valid_lens):
    queries = np.ascontiguousarray(queries, dtype=np.float32)
    keys = np.ascontiguousarray(keys, dtype=np.float32)
    values = np.ascontiguousarray(values, dtype=np.float32)
    valid_lens = np.asarray(valid_lens)
    assert queries.shape == (B, L, D), queries.shape
    vl = valid_lens.astype(np.int64)

    # Sort batches by valid_len descending; slot s on core c gets the batch
    # of rank s*8 + c.  Each slot's loop count covers the max valid_len in
    # its rank group, so one instruction stream fits all cores.
    order = np.argsort(-vl, kind="stable")
    counts = tuple(
        max(1, math.ceil(int(vl[order[s * N_CORES]]) / KB)) for s in range(SLOTS)
    )
    # Pairs share a K^T tile sized by the even slot; counts are descending.
    nc = _NC_CACHE.get(counts)
    if nc is None:
        nc = build_kernel(counts)
        _NC_CACHE[counts] = nc

    col = np.arange(L)
    in_maps = []
    for c in range(N_CORES):
        batch_idx = [int(order[s * N_CORES + c]) for s in range(SLOTS)]
        qt = np.ascontiguousarray(
            queries[batch_idx].transpose(0, 2, 1)
        )  # [SLOTS, D, L]
        kt = np.ascontiguousarray(keys[batch_idx].transpose(0, 2, 1))
        v = np.concatenate(
            [values[batch_idx], np.ones((SLOTS, L, 1), np.float32)], axis=2
        )
        bias = np.zeros((KB, SLOTS * N_KB), dtype=np.float32)
        for s in range(SLOTS):
            mask = (col >= vl[batch_idx[s]]).astype(np.float32) * NEG  # [L]
            bias[:, s * N_KB : (s + 1) * N_KB] = mask.reshape(N_KB, KB).T
        in_maps.append(
            {
                "qt": qt,
                "kt": kt,
                "v": v,
                "bias": bias,
                "ones": np.ones((1, D), np.float32),
            }
        )
    return nc, in_maps, order


def _unshard(res, order):
    out = np.empty((B, L, D), dtype=np.float32)
    for c in range(N_CORES):
        o = res.results[c]["out"]  # [SLOTS, D, L]
        for s in range(SLOTS):
            out[int(order[s * N_CORES + c])] = o[s].T
    return out


def kernel(queries, keys, values, valid_lens):
    nc, in_maps, order = _prepare(queries, keys, values, valid_lens)
    res = run_bass_kernel_spmd(nc, in_maps, core_ids=list(range(N_CORES)))
    return _unshard(res, order)


def trace_run(queries, keys, values, valid_lens):
    """Like kernel() but traced; returns BassKernelResults (for test.py)."""
    nc, in_maps, order = _prepare(queries, keys, values, valid_lens)
    res = run_bass_kernel_spmd(
        nc, in_maps, core_ids=list(range(N_CORES)), trace=True
    )
    res.full_output = _unshard(res, order)
    return res


# revision 28
# speedup vs baseline: 17.2197x; 1.0083x over previous
"""Masked dot-product attention (B=64, L=1024, D=64, fp32) on 8 NeuronCores.

Strategy (data-parallel over batch, per the sharding hint):
  - Batches are sorted by valid_len (descending) and dealt round-robin to the
    8 cores so every core gets one batch from each of 8 "rank groups"; the
    per-slot key-block loop count is baked at build time as the max over that
    slot's rank group.  Key blocks that are entirely masked are never computed.
  - Scores are computed transposed, S^T[k, q] = K @ Q^T, via
    matmul(lhsT=K^T_slice, rhs=Q^T) so that the softmax axis (k) lands on the
    partition dim.  Q and K are passed pre-transposed [D, L] per batch (host
    layout choice at shard time; there is no 4-byte DMA transpose on TRN2).
  - The sequence mask is fused into the exp: ScalarE computes
    P^T = exp(S^T/8 + bias_k) with a per-partition bias column that is 0 for
    valid keys and -1e6 for masked keys (exp underflows to exactly 0).
  - AV uses V augmented with a ones column: O'^T = [V | 1]^T @ P^T, so row 64
    of the accumulator is the softmax denominator for free.
  - Normalization: VectorE reciprocal of the denominator row, replicated
    across partitions by a step-0 free-dim SBUF->SBUF DMA (PE ones-matmul
    broadcast for the final slot, where latency matters), then one
    VectorE multiply.
All matmuls run in float32r (~1.2e-4 relative error, full PE rate).

Scheduling notes (the in-order engine streams make emission order matter):
  - kb loop is software-pipelined: QK(kb+1) is emitted before AV(kb) so PE
    never parks behind an AV that waits on ScalarE's exp.
  - Pair/slot input DMAs are prefetched one slot ahead; the first pair's
    loads are split so the first QK only waits on ~300KB.
  - The divide epilogue is deferred into the next slot's loop and split into
    independent q-halves to shorten the end-of-kernel serial chain.
"""

import math
from contextlib import ExitStack

import numpy as np

import concourse.bass as bass
import concourse.bacc as bacc
import concourse.mybir as mybir
import concourse.tile as tile
from concourse.bass_utils import run_bass_kernel_spmd

F32 = mybir.dt.float32
F32R = mybir.dt.float32r
EXP = mybir.ActivationFunctionType.Exp

B, L, D = 64, 1024, 64
N_CORES = 8
SLOTS = B // N_CORES  # batches per core
KB = 128              # key-block size (partition dim of S^T)
N_KB = L // KB        # max key blocks
QH = 512              # q chunk per matmul (fp32 moving-operand max)
NQH = L // QH
NEG = -1000000.0


def build_kernel(counts):
    """counts[s] = number of 128-wide key blocks to process for slot s."""
    nc = bacc.Bacc()

    qt_d = nc.dram_tensor("qt", [SLOTS, D, L], F32R, kind="ExternalInput")
    kt_d = nc.dram_tensor("kt", [SLOTS, D, L], F32R, kind="ExternalInput")
    v_d = nc.dram_tensor("v", [SLOTS, L, D + 1], F32R, kind="ExternalInput")
    bias_d = nc.dram_tensor("bias", [KB, SLOTS * N_KB], F32, kind="ExternalInput")
    out_d = nc.dram_tensor("out", [SLOTS, D, L], F32, kind="ExternalOutput")

    with tile.TileContext(nc) as tc, ExitStack() as ctx:
        const_pool = ctx.enter_context(tc.tile_pool(name="const", bufs=1))
        qk_pool = ctx.enter_context(tc.tile_pool(name="qk", bufs=3))
        v_pool = ctx.enter_context(tc.tile_pool(name="v", bufs=4))
        p_pool = ctx.enter_context(tc.tile_pool(name="p", bufs=4))
        ep_pool = ctx.enter_context(tc.tile_pool(name="ep", bufs=3))
        out_pool = ctx.enter_context(tc.tile_pool(name="out", bufs=2))
        psum_s = ctx.enter_context(tc.tile_pool(name="psum_s", bufs=2, space="PSUM"))
        psum_o = ctx.enter_context(tc.tile_pool(name="psum_o", bufs=2, space="PSUM"))

        bias_t = const_pool.tile([KB, SLOTS * N_KB], F32)
        warm_t = const_pool.tile([1, 1], F32)
        ones_t = const_pool.tile([1, D], F32R)

        pair_tiles: dict[int, tuple] = {}
        v_tiles: dict[int, object] = {}
        pair_order = [1, 2, 3, 0]  # big pair last: tail epilogues hide in its long loops
        slot_order = [2 * p + h for p in pair_order for h in range(2)]
        next_pair = {pair_order[i]: pair_order[i + 1] for i in range(len(pair_order) - 1)}
        next_slot = {slot_order[i]: slot_order[i + 1] for i in range(len(slot_order) - 1)}

        def load_pair(p):
            if p in pair_tiles:
                return
            n_max = counts[2 * p]
            # Two batches packed on the partition dim: even batch in
            # partitions 0-63, odd batch in 64-127.
            qt_t = qk_pool.tile([2 * D, L], F32R, tag="qt", name="qt_t")
            kt_t = qk_pool.tile([2 * D, L], F32R, tag="kt", name="kt_t")
            src_q = qt_d[2 * p : 2 * p + 2].rearrange("b d l -> (b d) l")
            src_k = kt_d[2 * p : 2 * p + 2].rearrange("b d l -> (b d) l")
            if not pair_tiles:
                # Piecewise: the first slot's kb-0 QKs only wait on the kt
                # head block + their own 64 qt rows (~320KB, 2 gens).
                nc.sync.dma_start(kt_t[:, :KB], src_k[:, :KB])
                nc.sync.dma_start(qt_t[:D, :], src_q[:D, :])
                nc.sync.dma_start(qt_t[D:, :], src_q[D:, :])
                if n_max > 1:
                    nc.sync.dma_start(
                        kt_t[:, KB : n_max * KB], src_k[:, KB : n_max * KB]
                    )
            else:
                nc.sync.dma_start(qt_t[:], src_q)
                nc.sync.dma_start(kt_t[:, : n_max * KB], src_k[:, : n_max * KB])
            pair_tiles[p] = (qt_t, kt_t)

        def load_v(s):
            if s in v_tiles:
                return
            n_kb = counts[s]
            v_t = v_pool.tile([KB, N_KB, D + 1], F32R, name="v_t")
            nc.gpsimd.dma_start(
                v_t[:, :n_kb, :],
                v_d[s].rearrange("(n p) d -> p n d", p=KB)[:, :n_kb, :],
            )
            v_tiles[s] = v_t

        def qk(s_ps, rows, kt_t, qt_t, kb):
            for qh in range(NQH):
                nc.tensor.matmul(
                    s_ps[:, qh * QH : (qh + 1) * QH],
                    kt_t[rows, kb * KB : (kb + 1) * KB],
                    qt_t[rows, qh * QH : (qh + 1) * QH],
                    start=True,
                    stop=True,
                )

        def make_tail(s, o_ps, rec_b, qh):
            # Deferred epilogue part B for one q-half: divide and store.
            c0, c1 = qh * QH, (qh + 1) * QH

            last = s == slot_order[-1]

            def tail():
                out_sb = out_pool.tile([D, QH], F32, name="out_sb")
                nch = 1
                cw = QH // nch
                for ch in range(nch):
                    nc.vector.tensor_tensor(
                        out_sb[:, ch * cw : (ch + 1) * cw],
                        o_ps[:D, c0 + ch * cw : c0 + (ch + 1) * cw],
                        rec_b[:, c0 + ch * cw : c0 + (ch + 1) * cw],
                        op=mybir.AluOpType.mult,
                    )
                    nc.sync.dma_start(
                        out_d[s][:, c0 + ch * cw : c0 + (ch + 1) * cw],
                        out_sb[:, ch * cw : (ch + 1) * cw],
                    )

            return tail

        load_pair(pair_order[0])
        # bias rides the SWDGE path so the first exp isn't queued behind
        # the HWDGE input loads.
        nc.gpsimd.dma_start(bias_t[:], bias_d[:])
        # Warm the exp table set while the first DMAs run; also build a
        # ones row (exp of 0 * bias) for the tail's PE broadcast.
        nc.scalar.activation(warm_t[:], bias_t[0:1, 0:1], EXP)
        nc.scalar.activation(ones_t[:], bias_t[0:1, :D], EXP, scale=0.0)
        load_v(slot_order[0])

        # Flat (slot, kb) work list, software-pipelined at depth 2 across
        # slot boundaries: the PE stream is QK(i+1), AV(i-1), so PE never
        # refills the pipeline at a slot change and AV only ever consumes
        # an exp that finished a full iteration ago.
        work = [(s, kb) for s in slot_order for kb in range(counts[s])]
        n_work = len(work)
        slot_first = {s: i for i, (s, kb) in reversed(list(enumerate(work)))}
        o_tiles: dict[int, object] = {}
        s_tiles: dict[tuple, object] = {}
        p_tiles: dict[tuple, object] = {}
        pending_tails: list = []
        tail_due: int = -1

        def emit_qk(i):
            s, kb = work[i]
            pair, half = divmod(s, 2)
            if kb == 0:
                # Slot prologue: prefetch upcoming inputs.
                nxt = slot_order.index(s) + 1
                if nxt < SLOTS:
                    load_v(slot_order[nxt])
                    if nxt + 1 < SLOTS:
                        load_v(slot_order[nxt + 1])
                if half == 0 and pair in next_pair:
                    load_pair(next_pair[pair])
                if half == 1 and pair in next_pair and next_pair[pair] in next_pair:
                    load_pair(next_pair[next_pair[pair]])
            qt_t, kt_t = pair_tiles[pair]
            rows = slice(D * half, D * half + D)
            s_tiles[(s, kb)] = psum_s.tile([KB, L], F32, tag="s", name="s_ps")
            qk(s_tiles[(s, kb)], rows, kt_t, qt_t, kb)

        def emit_av(i):
            s, kb = work[i]
            n_kb = counts[s]
            if kb == 0:
                o_tiles[s] = psum_o.tile([D + 1, L], F32, tag="o", name="o_ps")
            o_ps = o_tiles[s]
            p_t = p_tiles.pop((s, kb))
            for qh in range(NQH):
                nc.tensor.matmul(
                    o_ps[:, qh * QH : (qh + 1) * QH],
                    v_tiles[s][:, kb, :],
                    p_t[:, qh * QH : (qh + 1) * QH],
                    start=(kb == 0),
                    stop=(kb == n_kb - 1),
                )
            if kb == n_kb - 1:
                emit_epilogue_a(s)

        def emit_epilogue_a(s):
            # Reciprocal of the denominator row, then partition-replicate.
            nonlocal pending_tails, tail_due
            o_ps = o_tiles[s]
            last = s == slot_order[-1]
            rdt = F32R if last else F32
            rec_row = ep_pool.tile([1, L], rdt, tag="l", name="rec_row")
            rec_b = ep_pool.tile([D, L], rdt, tag="rec", name="rec_b")
            for qh in range(NQH):
                c0, c1 = qh * QH, (qh + 1) * QH
                with nc.allow_low_precision("f32r label for PE-broadcast tail"):
                    nc.vector.reciprocal(rec_row[:, c0:c1], o_ps[D : D + 1, c0:c1])
                if last:
                    # Tail: PE broadcast + ScalarE copy (both idle by now;
                    # keeps the serial DVE chain to recip + multiply).
                    bc_ps = psum_s.tile([D, QH], F32, tag="s", name="bc_ps")
                    nc.tensor.matmul(
                        bc_ps[:], ones_t[:], rec_row[:, c0:c1],
                        start=True, stop=True,
                    )
                    nc.scalar.copy(rec_b[:, c0:c1], bc_ps[:])
                else:
                    row_ap = rec_row[:, c0:c1]
                    bcast_src = bass.AP(
                        row_ap.tensor, row_ap.offset,
                        [list(row_ap.ap)[0], [0, D]] + list(row_ap.ap)[1:],
                    )
                    nc.gpsimd.dma_start(rec_b[:, c0:c1], bcast_src)
            pending_tails = [make_tail(s, o_ps, rec_b, qh) for qh in range(NQH)]
            tail_due = min(slot_first.get(slot_order[slot_order.index(s) + 1], 0) + 2
                           if slot_order.index(s) + 1 < SLOTS else 0, n_work - 1)

        emit_qk(0)
        for i in range(n_work):
            if i + 1 < n_work:
                emit_qk(i + 1)
            if pending_tails and i >= tail_due:
                for t in pending_tails:
                    t()
                pending_tails = []
            s, kb = work[i]
            p_tiles[(s, kb)] = p_pool.tile([KB, L], F32R, name="p_t")
            nc.scalar.activation(
                p_tiles[(s, kb)][:],
                s_tiles.pop((s, kb))[:],
                EXP,
                bias=bias_t[:, s * N_KB + kb : s * N_KB + kb + 1],
                scale=1.0 / math.sqrt(D),
            )
            if i >= 1:
                emit_av(i - 1)
        emit_av(n_work - 1)
        for t in pending_tails:
            t()

    nc.finalize()
    return nc


_NC_CACHE: dict[tuple, object] = {}


def _prepare(queries, keys, values, valid_lens):
    queries = np.ascontiguousarray(queries, dtype=np.float32)
    keys = np.ascontiguousarray(keys, dtype=np.float32)
    values = np.ascontiguousarray(values, dtype=np.float32)
    valid_lens = np.asarray(valid_lens)
    assert queries.shape == (B, L, D), queries.shape
    vl = valid_lens.astype(np.int64)

    # Sort batches by valid_len descending; slot s on core c gets the batch
    # of rank s*8 + c.  Each slot's loop count covers the max valid_len in
    # its rank group, so one instruction stream fits all cores.
    order = np.argsort(-vl, kind="stable")
    counts = tuple(
        max(1, math.ceil(int(vl[order[s * N_CORES]]) / KB)) for s in range(SLOTS)
    )
    # Pairs share a K^T tile sized by the even slot; counts are descending.
    nc = _NC_CACHE.get(counts)
    if nc is None:
        nc = build_kernel(counts)
        _NC_CACHE[counts] = nc

    col = np.arange(L)
    in_maps = []
    for c in range(N_CORES):
        batch_idx = [int(order[s * N_CORES + c]) for s in range(SLOTS)]
        qt = np.ascontiguousarray(
            queries[batch_idx].transpose(0, 2, 1)
        )  # [SLOTS, D, L]
        kt = np.ascontiguousarray(keys[batch_idx].transpose(0, 2, 1))
        v = np.concatenate(
            [values[batch_idx], np.ones((SLOTS, L, 1), np.float32)], axis=2
        )
        bias = np.zeros((KB, SLOTS * N_KB), dtype=np.float32)
        for s in range(SLOTS):
            mask = (col >= vl[batch_idx[s]]).astype(np.float32) * NEG  # [L]
            bias[:, s * N_KB : (s + 1) * N_KB] = mask.reshape(N_KB, KB).T
        in_maps.append({"qt": qt, "kt": kt, "v": v, "bias": bias})
    return nc, in_maps, order


def _unshard(res, order):
    out = np.empty((B, L, D), dtype=np.float32)
    for c in range(N_CORES):
        o = res.results[c]["out"]  # [SLOTS, D, L]
        for s in range(SLOTS):
            out[int(order[s * N_CORES + c])] = o[s].T
    return out


def kernel(queries, keys, values, valid_lens):
    nc, in_maps, order = _prepare(queries, keys, values, valid_lens)
    res = run_bass_kernel_spmd(nc, in_maps, core_ids=list(range(N_CORES)))
    return _unshard(res, order)


def trace_run(queries, keys, values, valid_lens):
    """Like kernel() but traced; returns BassKernelResults (for test.py)."""
    nc, in_maps, order = _prepare(queries, keys, values, valid_lens)
    res = run_bass_kernel_spmd(
        nc, in_maps, core_ids=list(range(N_CORES)), trace=True
    )
    res.full_output = _unshard(res, order)
    return res
